# revision 34
# baseline (speedup 1.0000x reference)
"""Griffin recurrence Trainium2 kernel.

Sharding: 8 cores = 4 batches x 2 channel-halves (192 channels each).
Layout on device: [channels, seq]. The projection matmul runs on the PE in
float32r (full PE rate at >=256 moving cols); all transcendentals use ONLY
the ln/exp activation table (sigma(z) = exp(-ln(1+exp(-z))), sqrt(x) =
exp(0.5 ln x), 1/clip(cd,1e-10) = exp(min(cum, 23.0259))), so the ACT
engine never swaps function tables. The chunked scan's cross-chunk pass is
stitched incrementally per 512-column block with AP-seeded scans, and the
incoming chunk state is folded into the chunk-start element of the w
sequence before the second scan, so there is no separate combine phase.

Per core:
  inputs  xb  [8*128, 16*512]  x[b].T packed block-major (see _pack_x)
          wT  [2048, 640]      packed W slice (see _pack_w), fp32r-rounded
          nbA [128, 1]         NEGATED decay bias, channels 0..127
          nbB [128, 1]         [-db for channels 128..191; zeros(64)]
  output  out [192, seq]       states, channel-major
"""

import sys

sys.path.insert(0, "/opt/trn_rl_repo")

from contextlib import ExitStack

import numpy as np

import bass_rust as _bass_rust

from concourse import bacc, mybir, tile
from concourse.bass_utils import run_bass_kernel_spmd
from concourse.hw_specs import get_activation_tables

f32 = mybir.dt.float32
f32r = mybir.dt.float32r
bf16 = mybir.dt.bfloat16
AF = mybir.ActivationFunctionType
ALU = mybir.AluOpType
AXL = mybir.AxisListType

D_MODEL = 2048
D_REC = 384
CHUNK = 64
NCORE = 8
CH = 192  # channels per core
BLK = 1024  # seq columns per pipeline block (= 2 PSUM banks)
NK = D_MODEL // 128  # 16 k-tiles
NT = 5  # M-tiles per core (640 = 5*128 packed W rows)
CPB = BLK // CHUNK  # chunks per block (16)
LN_EPS = 23.025850929940457  # -ln(1e-10)

_built = {}


class _Bacc(bacc.Bacc):
    """Bacc whose activation-table chooser is restricted to the one table
    holding every function this kernel uses (exp, ln, copy), so the ACT
    engine performs a single table load instead of swapping per call.
    Table list positions are preserved — `act_func_set_id` indexes
    act_info.json — only the candidate function sets are masked."""

    _ACT_TABLE = "natural_log_exp_and_others"

    def insert_act_table_loads(self):
        has_activation = any(
            isinstance(i, mybir.InstActivation)
            for b in self.main_func.blocks
            for i in b.instructions
        )
        if not has_activation:
            return
        tables = [
            (name, funcs if name == self._ACT_TABLE else set())
            for name, funcs in get_activation_tables(self.m.arch).items()
        ]
        _bass_rust.insert_act_table_loads(self, tables)


def _round_fp32r(a: np.ndarray) -> np.ndarray:
    """Round fp32 to fp32r (11 mantissa bits, low 12 bits zero), RNE."""
    u = np.ascontiguousarray(a, dtype=np.float32).view(np.uint32)
    rem = u & np.uint32(0xFFF)
    keep = u & np.uint32(0xFFFFF000)
    lsb = (u >> np.uint32(12)) & np.uint32(1)
    up = (rem > 0x800) | ((rem == 0x800) & (lsb == 1))
    return (keep + (up.astype(np.uint32) << np.uint32(12))).view(np.float32)


def _emit(tc, nc, xb, wT, nbA, nbB, out, seq):
    nblk = seq // BLK

    with ExitStack() as ctx:
        const = ctx.enter_context(tc.tile_pool(name="const", bufs=1))
        carry = ctx.enter_context(tc.tile_pool(name="carry", bufs=1))
        xp = ctx.enter_context(tc.tile_pool(name="xp", bufs=2))
        pp = ctx.enter_context(tc.tile_pool(name="pp", bufs=1, space="PSUM"))
        wk = ctx.enter_context(tc.tile_pool(name="wk", bufs=2))
        ob_pool = ctx.enter_context(tc.tile_pool(name="obp", bufs=2))

        # constants
        mask = const.tile([128, BLK], f32, tag="mask")
        nc.vector.memset(mask[:], 1.0)
        for c in range(CPB):
            nc.vector.memset(mask[:, c * CHUNK : c * CHUNK + 1], 0.0)
        ones8 = const.tile([128, CPB], f32, tag="ones8")
        nc.vector.memset(ones8[:], 1.0)
        nbA_t = const.tile([128, 1], f32, tag="nbA")
        nc.scalar.dma_start(nbA_t[:], nbA[:])
        nbB_t = const.tile([128, 1], f32, tag="nbB")
        nc.scalar.dma_start(nbB_t[:], nbB[:])
        # weight k-tiles on the scalar queue (idle at startup; x queues
        # stay clear so block 0's x lands first)
        wt = []
        for k in range(NK):
            w = const.tile([128, NT * 128], bf16, tag=f"wt{k}")
            nc.scalar.dma_start(w[:], wT[k * 128 : (k + 1) * 128, :])
            wt.append(w)

        # cross-block carried stitch state per group
        st_carry = {}
        for g in ("A", "B"):
            Lc = carry.tile([128, 1], f32, tag=f"Lc{g}")
            nc.vector.memset(Lc[:], 0.0)
            Wc = carry.tile([128, 1], f32, tag=f"Wc{g}")
            nc.vector.memset(Wc[:], 0.0)
            IS = carry.tile([128, 1], f32, tag=f"IS{g}")
            nc.vector.memset(IS[:], 0.0)
            st_carry[g] = (Lc, Wc, IS)

        def post_x(s0, w):
            """Post the x DMAs for the step at [s0, s0+w) on the sync and
            gpsimd queues (the gpsimd posts are emitted before that block's
            gpsimd compute so the queue serves them first)."""
            blk = s0 // BLK
            c0 = s0 - blk * BLK
            xk = []
            for k in range(NK):
                r0 = (blk * NK + k) * 128
                xt = xp.tile([128, BLK], bf16, tag=f"xk{k}", name=f"xk{k}")
                xq = nc.gpsimd if k % 4 == 3 else nc.sync
                xq.dma_start(
                    xt[:, 0:w], xb[r0 : r0 + 128, c0 : c0 + w]
                )
                xk.append(xt)
            return xk

        def emit_front(xk, s0, w, k_outer=False, tail=False, next_spec=None,
                       xoff=0):
            """Matmuls + next-step x prefetch + PSUM evacuation for the
            step at [s0, s0+w). The scan chain is emitted separately (one
            step later) so the next step's evacuation — which frees PSUM
            banks for the PE — is never queued behind chain compute on the
            ACT FIFO. Returns the context dict for emit_chain.
            k_outer: per-k matmul groups (PE streams as DMAs land; block 0).
            tail: run the multiplies on DVE and the out DMA on sync to
            minimize post-matmul latency (final sub-blocks)."""
            cpb = w // CHUNK

            ps = []
            for t in range(NT):
                p = pp.tile([128, BLK], f32, tag=f"ps{t % 4}", bufs=1)
                ps.append(p)
            # matmul outputs may not span a PSUM bank boundary: emit one
            # matmul per 512-column half
            HB = 512
            nh = (w + HB - 1) // HB
            if k_outer:
                for k in range(NK):
                    for t in range(NT):
                        for hh in range(nh):
                            o0 = hh * HB
                            o1 = min(w, o0 + HB)
                            nc.tensor.matmul(
                                ps[t][:, o0:o1],
                                wt[k][:, t * 128 : (t + 1) * 128],
                                xk[k][:, xoff + o0 : xoff + o1],
                                start=(k == 0),
                                stop=(k == NK - 1),
                                skip_group_check=True,
                            )
            else:
                for t in range(NT):
                    for k in range(NK):
                        for hh in range(nh):
                            o0 = hh * HB
                            o1 = min(w, o0 + HB)
                            nc.tensor.matmul(
                                ps[t][:, o0:o1],
                                wt[k][:, t * 128 : (t + 1) * 128],
                                xk[k][:, xoff + o0 : xoff + o1],
                                start=(k == 0),
                                stop=(k == NK - 1),
                                skip_group_check=True,
                            )

            next_xk = post_x(*next_spec) if next_spec else None

            # PSUM evacuation, ln/exp table only (Copy is table-free).
            # Ln runs in place on the Exp result.
            ma = wk.tile([128, BLK], f32, tag="ma", bufs=2)
            nc.scalar.activation(
                ma[:, 0:w], ps[0][:, 0:w], AF.Exp, bias=nbA_t[:], scale=-1.0
            )
            nc.scalar.activation(ma[:, 0:w], ma[:, 0:w], AF.Ln, bias=1.0)
            mi = wk.tile([128, BLK], f32, tag="mi", bufs=2)
            nc.scalar.activation(mi[:, 0:w], ps[1][:, 0:w], AF.Exp, scale=-1.0)
            nc.scalar.activation(mi[:, 0:w], mi[:, 0:w], AF.Ln, bias=1.0)
            vA = wk.tile([128, BLK], f32, tag="vA", bufs=2)
            nc.scalar.activation(vA[:, 0:w], ps[2][:, 0:w], AF.Copy)
            mb = wk.tile([128, BLK], f32, tag="mb", bufs=2)
            nc.scalar.activation(
                mb[:, 0:w], ps[3][:, 0:w], AF.Exp, bias=nbB_t[:], scale=-1.0
            )
            nc.scalar.activation(mb[:, 0:w], mb[:, 0:w], AF.Ln, bias=1.0)
            vB = wk.tile([64, BLK], f32, tag="vB", bufs=2)
            nc.scalar.activation(vB[:, 0:w], ps[4][0:64, 0:w], AF.Copy)
            # realign i_B's m to partitions 0..63 (sync queue: idle)
            mbi = wk.tile([64, BLK], f32, tag="mbi", bufs=2)
            nc.sync.dma_start(mbi[:, 0:w], mb[64:128, 0:w])

            return dict(
                s0=s0, w=w, cpb=cpb, tail=tail, next_xk=next_xk,
                ma=ma, mi=mi, vA=vA, mb=mb, vB=vB, mbi=mbi,
            )

        def emit_chain(c):
            """Scan chain for a step whose front was emitted earlier."""
            s0, w, cpb, tail = c["s0"], c["w"], c["cpb"], c["tail"]
            ma, mi, vA, mb, vB, mbi = (
                c["ma"], c["mi"], c["vA"], c["mb"], c["vB"], c["mbi"]
            )
            mule = nc.vector if tail else nc.gpsimd
            oute = nc.sync if tail else nc.gpsimd

            # per-group tiles; stages below are emitted interleaved A/B so
            # the two independent chains pipeline through the engines
            G = []
            for name, pg, m_t, mi_t, v_t, orow in (
                ("A", 128, ma, mi, vA, 0),
                ("B", 64, mb, mbi, vB, 128),
            ):
                g = {
                    "pg": pg, "m": m_t[0:pg, 0:w], "mi": mi_t, "v": v_t,
                    "orow": orow, "carry": st_carry[name],
                }
                g["cum"] = wk.tile([pg, BLK], f32, tag=f"cum{name}", name=f"cum{name}", bufs=1)
                g["cd"] = wk.tile([pg, BLK], f32, tag=f"cd{name}", name=f"cd{name}", bufs=1)
                g["a2"] = wk.tile([pg, BLK], f32, tag=f"a2{name}", name=f"a2{name}", bufs=1)
                g["u"] = wk.tile([pg, BLK], f32, tag=f"u{name}", name=f"u{name}", bufs=1)
                g["sw"] = wk.tile([pg, BLK], f32, tag=f"sw{name}", name=f"sw{name}", bufs=1)
                g["wsum"] = wk.tile([pg, CPB], f32, tag=f"wsum{name}", name=f"wsum{name}")
                g["Mc8"] = wk.tile([pg, CPB], f32, tag=f"Mc8{name}", name=f"Mc8{name}")
                g["Lam8"] = wk.tile([pg, CPB], f32, tag=f"Lam8{name}", name=f"Lam8{name}")
                g["CD8"] = wk.tile([pg, CPB], f32, tag=f"CD8{name}", name=f"CD8{name}")
                g["LamC8"] = wk.tile([pg, CPB], f32, tag=f"LamC8{name}", name=f"LamC8{name}")
                g["iCD8"] = wk.tile([pg, CPB], f32, tag=f"iCD8{name}", name=f"iCD8{name}")
                g["F8"] = wk.tile([pg, CPB], f32, tag=f"F8{name}", name=f"F8{name}")
                g["t8"] = wk.tile([pg, CPB], f32, tag=f"t8{name}", name=f"t8{name}")
                g["CW8"] = wk.tile([pg, CPB], f32, tag=f"CW8{name}", name=f"CW8{name}")
                g["inc8"] = wk.tile([pg, CPB], f32, tag=f"inc8{name}", name=f"inc8{name}")
                g["ob"] = ob_pool.tile([pg, BLK], f32, tag=f"ob{name}", name=f"ob{name}")
                G.append(g)

            for g in G:
                pg = g["pg"]
                nc.vector.tensor_tensor_scan(
                    g["cum"][:, 0:w], mask[0:pg, 0:w], g["m"], 0.0,
                    ALU.mult, ALU.add,
                )
            for g in G:
                nc.scalar.activation(
                    g["cd"][:, 0:w], g["cum"][:, 0:w], AF.Exp, scale=-1.0
                )
            for g in G:
                # boundary -log-decay (reads cum before the clip below)
                nc.vector.tensor_scalar_min(
                    g["Mc8"][:, 0:cpb], g["cum"][:, CHUNK - 1 : w : CHUNK], LN_EPS
                )
            for g in G:
                nc.vector.tensor_scalar_min(
                    g["cum"][:, 0:w], g["cum"][:, 0:w], LN_EPS
                )
            for g in G:
                nc.scalar.activation(g["a2"][:, 0:w], g["m"], AF.Exp, scale=-2.0)
            for g in G:
                nc.scalar.activation(
                    g["a2"][:, 0:w], g["a2"][:, 0:w], AF.Ln, bias=1.0, scale=-1.0
                )
            # gate exponent: 0.5*ln(1-a2) - m_i + min(cum, LN_EPS); the
            # exp of it IS sqrt(1-a^2)*i/clip(cum_decay), so one multiply
            # by v yields w directly (no reciprocal, no separate inv pass)
            for g in G:
                pg = g["pg"]
                nc.vector.tensor_sub(
                    g["mi"][0:pg, 0:w], g["cum"][:, 0:w], g["mi"][0:pg, 0:w]
                )
                nc.vector.scalar_tensor_tensor(
                    g["a2"][:, 0:w], g["a2"][:, 0:w], 0.5, g["mi"][0:pg, 0:w],
                    ALU.mult, ALU.add,
                )
            for g in G:
                nc.scalar.activation(g["a2"][:, 0:w], g["a2"][:, 0:w], AF.Exp)
            for g in G:
                pg = g["pg"]
                mule.tensor_mul(g["u"][:, 0:w], g["a2"][:, 0:w], g["v"][0:pg, 0:w])
            for g in G:
                nc.vector.tensor_reduce(
                    g["wsum"][:, 0:cpb],
                    g["u"][:, 0:w].rearrange("p (c s) -> p c s", c=cpb),
                    AXL.X,
                    ALU.add,
                )
            # incremental cross-chunk stitch
            for g in G:
                pg = g["pg"]
                Lc, Wc, IS = g["carry"]
                nc.vector.tensor_tensor_scan(
                    g["Lam8"][:, 0:cpb], ones8[0:pg, 0:cpb], g["Mc8"][:, 0:cpb],
                    Lc[0:pg, :], ALU.mult, ALU.add,
                )
                nc.vector.tensor_copy(Lc[0:pg, :], g["Lam8"][:, cpb - 1 : cpb])
            for g in G:
                nc.scalar.activation(
                    g["CD8"][:, 0:cpb], g["Lam8"][:, 0:cpb], AF.Exp, scale=-1.0
                )
            for g in G:
                nc.vector.tensor_scalar_min(
                    g["LamC8"][:, 0:cpb], g["Lam8"][:, 0:cpb], LN_EPS
                )
            for g in G:
                nc.scalar.activation(
                    g["iCD8"][:, 0:cpb], g["LamC8"][:, 0:cpb], AF.Exp
                )
            for g in G:
                nc.vector.tensor_mul(
                    g["F8"][:, 0:cpb], g["cd"][:, CHUNK - 1 : w : CHUNK],
                    g["wsum"][:, 0:cpb],
                )
                nc.vector.tensor_mul(
                    g["t8"][:, 0:cpb], g["F8"][:, 0:cpb], g["iCD8"][:, 0:cpb]
                )
            for g in G:
                pg = g["pg"]
                Lc, Wc, IS = g["carry"]
                nc.vector.tensor_tensor_scan(
                    g["CW8"][:, 0:cpb], ones8[0:pg, 0:cpb], g["t8"][:, 0:cpb],
                    Wc[0:pg, :], ALU.mult, ALU.add,
                )
                nc.vector.tensor_copy(Wc[0:pg, :], g["CW8"][:, cpb - 1 : cpb])
                nc.vector.tensor_copy(g["inc8"][:, 0:1], IS[0:pg, :])
                if cpb > 1:
                    nc.vector.tensor_mul(
                        g["inc8"][:, 1:cpb], g["CD8"][:, 0 : cpb - 1],
                        g["CW8"][:, 0 : cpb - 1],
                    )
                nc.vector.tensor_mul(
                    IS[0:pg, :], g["CD8"][:, cpb - 1 : cpb],
                    g["CW8"][:, cpb - 1 : cpb],
                )
            for g in G:
                # fold incoming state into chunk-start w, then scan
                nc.vector.tensor_add(
                    g["u"][:, 0:w:CHUNK], g["u"][:, 0:w:CHUNK], g["inc8"][:, 0:cpb]
                )
            for g in G:
                pg = g["pg"]
                nc.vector.tensor_tensor_scan(
                    g["sw"][:, 0:w], mask[0:pg, 0:w], g["u"][:, 0:w], 0.0,
                    ALU.mult, ALU.add,
                )
            for g in G:
                mule.tensor_mul(g["ob"][:, 0:w], g["cd"][:, 0:w], g["sw"][:, 0:w])
            for g in G:
                pg, orow = g["pg"], g["orow"]
                oute.dma_start(
                    out[orow : orow + pg, s0 : s0 + w], g["ob"][:, 0:w]
                )

        # block 0 streams matmuls per k as its DMAs land; the final block
        # runs in two half-width steps with a low-latency tail chain.
        # chain(b-1) is emitted after front(b): the ACT FIFO serves the
        # next step's PSUM evacuation before the previous step's chain.
        h = BLK // 2
        last = (nblk - 1) * BLK
        # (s0, w, k_outer, tail, xoff, x post spec for THIS step)
        plan = [(0, BLK, True, False, 0, (0, BLK))]
        plan += [
            (b * BLK, BLK, False, False, 0, (b * BLK, BLK))
            for b in range(1, nblk - 1)
        ]
        # final block: one full-width contiguous x post, two compute steps
        plan += [
            (last, h, False, True, 0, (last, BLK)),
            (last + h, h, False, True, h, None),
        ]
        xk = post_x(*plan[0][5])
        pending = []
        for i, (s0, w, ko, tail, xoff, _post) in enumerate(plan):
            nxt = plan[i + 1][5] if i + 1 < len(plan) else None
            c = emit_front(
                xk, s0, w, k_outer=ko, tail=tail, next_spec=nxt, xoff=xoff
            )
            if c["next_xk"] is not None:
                xk = c["next_xk"]
            pending.append(c)
            # keep one chain pending normally; two across the tail steps so
            # both tail evacuations reach the ACT FIFO before the previous
            # chain's compute
            while len(pending) > 1:
                emit_chain(pending.pop(0))
        while pending:
            emit_chain(pending.pop(0))


def _build(seq):
    if seq in _built:
        return _built[seq]
    nc = _Bacc(
        "TRN2", target_bir_lowering=False, debug=False, num_devices=NCORE
    )
    nblk = seq // BLK
    xb = nc.dram_tensor(
        "xb", [nblk * NK * 128, BLK], bf16, kind="ExternalInput"
    ).ap()
    wT = nc.dram_tensor("wT", [D_MODEL, NT * 128], bf16, kind="ExternalInput").ap()
    nbA = nc.dram_tensor("nbA", [128, 1], f32, kind="ExternalInput").ap()
    nbB = nc.dram_tensor("nbB", [128, 1], f32, kind="ExternalInput").ap()
    out = nc.dram_tensor("out", [CH, seq], f32, kind="ExternalOutput").ap()
    with tile.TileContext(nc) as tc:
        _emit(tc, nc, xb, wT, nbA, nbB, out, seq)
    nc.compile()
    _built[seq] = nc
    return nc


def _pack_w(W, h):
    """Pack this half's W rows into 640 rows of 5 M-tiles.

    t0 = a[0:128], t1 = i[0:128], t2 = v[0:128],
    t3 = [a[128:192]; i[128:192]], t4 = [v[128:192]; zeros]."""
    c0 = h * CH
    z = np.zeros((64, W.shape[1]), np.float32)
    return np.concatenate(
        [
            W[c0 : c0 + 128],
            W[D_REC + c0 : D_REC + c0 + 128],
            W[2 * D_REC + c0 : 2 * D_REC + c0 + 128],
            W[c0 + 128 : c0 + 192],
            W[D_REC + c0 + 128 : D_REC + c0 + 192],
            W[2 * D_REC + c0 + 128 : 2 * D_REC + c0 + 192],
            z,
        ],
        axis=0,
    )


def _bf16(a: np.ndarray) -> np.ndarray:
    """fp32 -> bf16 (RNE), returned as an ml_dtypes.bfloat16 array."""
    import ml_dtypes

    return np.ascontiguousarray(a, dtype=np.float32).astype(ml_dtypes.bfloat16)


def _pack_x(xb):
    """x[b] [seq, 2048] -> [nblk*16*128, 512]: per (block, k-tile) a fully
    contiguous [128, 512] slab, bf16."""
    seq = xb.shape[0]
    nblk = seq // BLK
    xr = xb.reshape(nblk, BLK, NK, 128).transpose(0, 2, 3, 1)
    return _bf16(np.ascontiguousarray(xr).reshape(nblk * NK * 128, BLK))


def _in_maps(x, W, db):
    maps = []
    xbs = {}
    for core in range(NCORE):
        b, h = core // 2, core % 2
        if b not in xbs:
            xbs[b] = _pack_x(x[b])
        c0 = h * CH
        wTc = _bf16(np.ascontiguousarray(_pack_w(W, h).T))
        nbAv = np.ascontiguousarray((-db[c0 : c0 + 128]).reshape(128, 1))
        nbBv = np.ascontiguousarray(
            np.concatenate(
                [-db[c0 + 128 : c0 + 192], np.zeros(64, np.float32)]
            ).reshape(128, 1)
        )
        maps.append({"xb": xbs[b], "wT": wTc, "nbA": nbAv, "nbB": nbBv})
    return maps


def kernel(x, W, decay_bias, _trace=False):
    x = np.asarray(x, np.float32)
    W = np.asarray(W, np.float32)
    db = np.asarray(decay_bias, np.float32)
    B, S, _ = x.shape
    nc = _build(S)
    res = run_bass_kernel_spmd(nc, _in_maps(x, W, db), list(range(NCORE)), trace=_trace)
    outf = np.empty((B, S, D_REC), np.float32)
    for core in range(NCORE):
        b, h = core // 2, core % 2
        outf[b, :, h * CH : (h + 1) * CH] = res.results[core]["out"].T
    if _trace:
        return outf, res
    return outf
